# revision 19
# baseline (speedup 1.0000x reference)
"""GRU encoder (embed -> GRU -> layernorm) Trainium2 Bass kernel.

Sharding: data-parallel over batch B=32 across 8 cores (4 rows each).
No collectives; each core runs an independent program on its batch slice.

Per-core pipeline:
  1. indirect-DMA gather of emb rows for the 2048 (b,s) tokens
  2. PE-transpose gathered rows + W_ih + W_hh into E/H-major bf16 layouts
  3. input GEMM xg = x_emb @ W_ih.T + biases  (bf16, gate-permuted cols r|n|z)
  4. 512 sequential GRU steps: hg = h @ W_hh.T via moving-W bf16 matmuls
     (lhsT = hT [128,4] chunks, rhs = W_hhT [128,512] chunks), gates on
     ACT/DVE, z/n transposed back to H-major via tiny PE transposes so the
     next step's lhsT needs no extra work
  5. final layernorm pass over the stored hidden states + output DMA
"""

import sys

sys.path.insert(0, "/opt/trn_rl_repo")

import numpy as np

import concourse.bass as bass
import concourse.tile as tile
from concourse import bacc, mybir
from concourse.masks import make_identity

F32 = mybir.dt.float32
BF16 = mybir.dt.bfloat16
I32 = mybir.dt.int32
AF = mybir.ActivationFunctionType
ALU = mybir.AluOpType

VOCAB, E, H = 32000, 512, 1024
B_FULL, S_FULL = 32, 512
N_CORES = 8
B = B_FULL // N_CORES  # 4 rows per core
G = 3 * H  # 3072 gate columns

# permuted gate-column layout: [r (0:1024) | n (1024:2048) | z (2048:3072)]
# (z last so the critical tail of each step is as short as possible)
R0, N0, Z0 = 0, H, 2 * H


def _perm_gchunk(c):
    """Map W row-chunk c (128 rows, original r,z,n order) to permuted col."""
    if c < 8:
        return 128 * c  # r
    if c < 16:
        return Z0 + 128 * (c - 8)  # z
    return N0 + 128 * (c - 16)  # n


# per 512-wide GEMM output chunk (original col order) -> permuted col base
_PERM_NCHUNK = [0, 512, Z0, Z0 + 512, N0, N0 + 512]


def build_kernel(S=S_FULL):
    nc = bacc.Bacc("TRN2", target_bir_lowering=False, debug=False,
                   enable_asserts=False)

    x_h = nc.dram_tensor("x", [B, S], I32, kind="ExternalInput")
    emb_h = nc.dram_tensor("emb", [VOCAB, E], F32, kind="ExternalInput")
    wih_h = nc.dram_tensor("W_ih", [G, E], F32, kind="ExternalInput")
    whh_h = nc.dram_tensor("W_hh", [G, H], F32, kind="ExternalInput")
    bih_h = nc.dram_tensor("b_ih", [G], F32, kind="ExternalInput")
    bhh_h = nc.dram_tensor("b_hh", [G], F32, kind="ExternalInput")
    gam_h = nc.dram_tensor("gamma", [H], F32, kind="ExternalInput")
    bet_h = nc.dram_tensor("beta", [H], F32, kind="ExternalInput")

    out_h = nc.dram_tensor("out", [B, S, H], F32, kind="ExternalOutput")
    st_h = nc.dram_tensor("state", [B, H], F32, kind="ExternalOutput")

    xg_h = nc.dram_tensor("xg_scratch", [B, S, G], BF16, kind="Internal")
    hraw_h = nc.dram_tensor("h_rawT", [S, 128, 4 * (H // 128)], BF16,
                            kind="Internal")

    NROWS = B * S          # (b, s) rows this core owns
    NRT = NROWS // 128     # 128-row tiles
    KE = E // 128          # 4  E-chunks
    KH = H // 128          # 8  H-chunks
    NG = G // 512          # 6  gate 512-chunks

    with tile.TileContext(nc) as tc:
        from contextlib import ExitStack

        with ExitStack() as ctx:
            const = ctx.enter_context(tc.tile_pool(name="const", bufs=1))

            # identities for PE transposes
            id_f32 = const.tile([128, 128], F32, tag="idf")
            make_identity(nc, id_f32[:])
            id_bf16 = const.tile([128, 128], BF16, tag="idb")
            nc.vector.tensor_copy(id_bf16[:], id_f32[:])

            # ones [1, B] bf16 for broadcasting b_hh_n into PSUM via matmul
            ones_b = const.tile([1, B], BF16, tag="ones")
            nc.gpsimd.memset(ones_b[:], 1.0)

            # biases: bias_perm[g'] = b_ih+b_hh (r,z) or b_ih (n), permuted
            braw = const.tile([1, 2 * G], F32, tag="braw")
            nc.sync.dma_start(braw[:, 0:G],
                              bih_h.ap().rearrange("(a g) -> a g", a=1))
            nc.sync.dma_start(braw[:, G:2 * G],
                              bhh_h.ap().rearrange("(a g) -> a g", a=1))
            bsum = const.tile([1, G], F32, tag="bsum")
            nc.vector.tensor_tensor(bsum[:], braw[:, 0:G], braw[:, G:2 * G],
                                    op=ALU.add)
            bperm = const.tile([1, G], F32, tag="bperm")
            nc.vector.tensor_copy(bperm[:, R0:R0 + H], bsum[:, 0:H])
            nc.vector.tensor_copy(bperm[:, Z0:Z0 + H], bsum[:, H:2 * H])
            nc.vector.tensor_copy(bperm[:, N0:N0 + H], braw[:, 2 * H:3 * H])
            # b_hh for the n gate (goes inside r*(...) so kept separate)
            bhhn = const.tile([1, H], BF16, tag="bhhn")
            nc.vector.tensor_copy(bhhn[:], braw[:, G + 2 * H:G + 3 * H])
            # broadcast bias across 128 partitions for the GEMM epilogue
            bias_bc = const.tile([128, G], F32, tag="biasbc")
            nc.gpsimd.partition_broadcast(bias_bc[:], bperm[:])

            # big persistent weight tile: W_hhT (H-major, permuted cols), bf16
            whhT = const.tile([128, KH * G], BF16, tag="whhT")

            # ---------------- phase 1: gather + transposes + GEMM ----------
            with ExitStack() as p1:
                gpool = p1.enter_context(tc.tile_pool(name="gath", bufs=3))
                tpsum = p1.enter_context(
                    tc.tile_pool(name="tpsum", bufs=4, space="PSUM"))
                big = p1.enter_context(tc.tile_pool(name="big", bufs=1))

                # token indices: idx[p, j] = x_flat[j*128 + p]
                idx = big.tile([128, NRT], I32, tag="idx")
                nc.gpsimd.dma_start(
                    idx[:],
                    x_h.ap().rearrange("b s -> (b s)").rearrange(
                        "(j p) -> p j", p=128))

                xembT = big.tile([128, KE * NROWS], BF16, tag="xembT")
                for j in range(NRT):
                    gt = gpool.tile([128, E], F32, tag="gt")
                    nc.gpsimd.indirect_dma_start(
                        out=gt[:],
                        out_offset=None,
                        in_=emb_h.ap(),
                        in_offset=bass.IndirectOffsetOnAxis(
                            ap=idx[:, j:j + 1], axis=0),
                    )
                    for e in range(KE):
                        pt = tpsum.tile([128, 128], F32, tag="pt")
                        nc.tensor.transpose(pt[:], gt[:, 128 * e:128 * e + 128],
                                            id_f32[:])
                        nc.vector.tensor_copy(
                            xembT[:, NROWS * e + 128 * j:
                                  NROWS * e + 128 * j + 128], pt[:])

                # W_ihT (E-major), bf16, original col order
                wihT = big.tile([128, KE * G], BF16, tag="wihT")
                for c in range(G // 128):
                    wt = gpool.tile([128, E], F32, tag="wt")
                    nc.sync.dma_start(wt[:], wih_h[128 * c:128 * c + 128, :])
                    for e in range(KE):
                        pt = tpsum.tile([128, 128], F32, tag="pt")
                        nc.tensor.transpose(pt[:], wt[:, 128 * e:128 * e + 128],
                                            id_f32[:])
                        nc.vector.tensor_copy(
                            wihT[:, G * e + 128 * c:G * e + 128 * c + 128],
                            pt[:])

                # GEMM: xg[rows, g] = x_emb @ W_ih.T  (+bias, permuted cols)
                gempsum = p1.enter_context(
                    tc.tile_pool(name="gemmps", bufs=4, space="PSUM"))
                xgout = p1.enter_context(tc.tile_pool(name="xgout", bufs=4))
                xg_rows = xg_h.ap().rearrange("b s g -> (b s) g")
                for rt in range(NRT):
                    for n in range(NG):
                        ps = gempsum.tile([128, 512], F32, tag="ps")
                        for e in range(KE):
                            nc.tensor.matmul(
                                ps[:],
                                lhsT=xembT[:, NROWS * e + 128 * rt:
                                           NROWS * e + 128 * rt + 128],
                                rhs=wihT[:, G * e + 512 * n:G * e + 512 * n + 512],
                                start=(e == 0), stop=(e == KE - 1))
                        pc = _PERM_NCHUNK[n]
                        xo = xgout.tile([128, 512], BF16, tag="xo")
                        nc.vector.tensor_tensor(
                            xo[:], ps[:], bias_bc[:, pc:pc + 512], op=ALU.add)
                        nc.sync.dma_start(
                            xg_rows[128 * rt:128 * rt + 128, pc:pc + 512],
                            xo[:])

                # W_hhT: H-major, permuted gate cols, bf16
                for c in range(G // 128):
                    wt2 = gpool.tile([128, H], F32, tag="wt2")
                    nc.sync.dma_start(wt2[:], whh_h[128 * c:128 * c + 128, :])
                    pcol = _perm_gchunk(c)
                    for k in range(KH):
                        pt = tpsum.tile([128, 128], F32, tag="pt")
                        nc.tensor.transpose(pt[:],
                                            wt2[:, 128 * k:128 * k + 128],
                                            id_f32[:])
                        nc.vector.tensor_copy(
                            whhT[:, G * k + pcol:G * k + pcol + 128], pt[:])

            # ---------------- phase 2: recurrence -------------------------
            with ExitStack() as p2:
                xgp = p2.enter_context(tc.tile_pool(name="xgp", bufs=3))
                prep = p2.enter_context(tc.tile_pool(name="prep", bufs=6))
                rznp = p2.enter_context(tc.tile_pool(name="rznp", bufs=2))
                htp = p2.enter_context(tc.tile_pool(name="htp", bufs=3))
                dep = p2.enter_context(tc.tile_pool(name="dep", bufs=3))
                psg = p2.enter_context(
                    tc.tile_pool(name="psg", bufs=6, space="PSUM"))
                pst = p2.enter_context(
                    tc.tile_pool(name="pst", bufs=2, space="PSUM"))

                hT = htp.tile([128, 4 * KH], BF16, tag="hT")
                nc.gpsimd.memset(hT[:], 0.0)

                for t in range(S):
                    xg = xgp.tile([B, G], BF16, tag="xg")
                    nc.sync.dma_start(
                        xg[:],
                        xg_h[:, t:t + 1, :].rearrange("b one g -> b (one g)"))

                    # hg psum chunks; col order r r n n z z (permuted layout)
                    ps = []
                    for n in range(NG):
                        p = psg.tile([B, 512], F32, tag="hgps")
                        ps.append(p)
                        first = True
                        if 512 * n >= N0 and 512 * n < Z0:
                            # n-gate chunk: seed with b_hh_n broadcast
                            off = 512 * n - N0
                            nc.tensor.matmul(
                                p[:], lhsT=ones_b[:],
                                rhs=bhhn[:, off:off + 512],
                                start=True, stop=False)
                            first = False
                        for k in range(KH):
                            nc.tensor.matmul(
                                p[:], lhsT=hT[:, 4 * k:4 * k + 4],
                                rhs=whhT[:, G * k + 512 * n:
                                         G * k + 512 * n + 512],
                                start=first and k == 0, stop=(k == KH - 1))

                    # r = sigmoid(hg_r + xg_r)   [B, 1024] bf16
                    r_sb = rznp.tile([B, H], BF16, tag="r")
                    for i in range(2):
                        pre = prep.tile([B, 512], F32, tag="pre")
                        nc.vector.tensor_tensor(
                            pre[:], ps[i][:], xg[:, R0 + 512 * i:R0 + 512 * i + 512],
                            op=ALU.add)
                        nc.scalar.activation(r_sb[:, 512 * i:512 * i + 512],
                                             pre[:], AF.Sigmoid)

                    # n = tanh(xg_n + r * (hg_n + b_hh_n))
                    n_sb = rznp.tile([B, H], BF16, tag="n")
                    for i in range(2):
                        t1 = prep.tile([B, 512], F32, tag="t1")
                        nc.vector.tensor_tensor(
                            t1[:], r_sb[:, 512 * i:512 * i + 512], ps[2 + i][:],
                            op=ALU.mult)
                        t2 = prep.tile([B, 512], F32, tag="t2")
                        nc.vector.tensor_tensor(
                            t2[:], t1[:], xg[:, N0 + 512 * i:N0 + 512 * i + 512],
                            op=ALU.add)
                        nc.scalar.activation(n_sb[:, 512 * i:512 * i + 512],
                                             t2[:], AF.Tanh)

                    # z = sigmoid(hg_z + xg_z)
                    z_sb = rznp.tile([B, H], BF16, tag="z")
                    for i in range(2):
                        pre = prep.tile([B, 512], F32, tag="prez")
                        nc.vector.tensor_tensor(
                            pre[:], ps[4 + i][:], xg[:, Z0 + 512 * i:Z0 + 512 * i + 512],
                            op=ALU.add)
                        nc.scalar.activation(z_sb[:, 512 * i:512 * i + 512],
                                             pre[:], AF.Sigmoid)

                    # transpose z, n back to H-major [128, 4*KH] (one shared
                    # psum bank: z in cols 0:32, n in cols 32:64)
                    psZN = pst.tile([128, 8 * KH], BF16, tag="psZN")
                    for c in range(KH):
                        nc.tensor.transpose(psZN[:, 4 * c:4 * c + 4],
                                            z_sb[:, 128 * c:128 * c + 128],
                                            id_bf16[0:B, 0:B])
                        nc.tensor.transpose(
                            psZN[:, 4 * KH + 4 * c:4 * KH + 4 * c + 4],
                            n_sb[:, 128 * c:128 * c + 128],
                            id_bf16[0:B, 0:B])
                    zT = dep.tile([128, 4 * KH], BF16, tag="zT")
                    nc.vector.tensor_copy(zT[:], psZN[:, 0:4 * KH])
                    nT = dep.tile([128, 4 * KH], BF16, tag="nT")
                    nc.scalar.copy(nT[:], psZN[:, 4 * KH:8 * KH])

                    # h' = n + z*(h - n)   (H-major)
                    d = dep.tile([128, 4 * KH], F32, tag="d")
                    nc.vector.tensor_tensor(d[:], hT[:], nT[:], op=ALU.subtract)
                    e_t = dep.tile([128, 4 * KH], F32, tag="e")
                    nc.vector.tensor_tensor(e_t[:], zT[:], d[:], op=ALU.mult)
                    hT = htp.tile([128, 4 * KH], BF16, tag="hT")
                    nc.vector.tensor_tensor(hT[:], nT[:], e_t[:], op=ALU.add)

                    nc.sync.dma_start(
                        hraw_h[t:t + 1, :, :].rearrange("one p f -> p (one f)"),
                        hT[:])

                # final state: strided DRAM->DRAM cast from h_rawT[S-1]
                # st[b, 128k+p] = hraw[S-1, p, 4k+b]
                with nc.allow_non_contiguous_dma(reason="one-time 8KB state"):
                    for b in range(B):
                        nc.gpsimd.dma_start(
                            st_h[b:b + 1, :].rearrange(
                                "one (k p) -> (one p) k", p=128),
                            hraw_h[S - 1:S, :, :].rearrange(
                                "one p (k b) -> (one p) k b", b=B)[:, :, b])

            # ---------------- phase 3: layernorm + output ------------------
            with ExitStack() as p3:
                lnin = p3.enter_context(tc.tile_pool(name="lnin", bufs=2))
                lnt = p3.enter_context(tc.tile_pool(name="lnt", bufs=2))
                lnsm = p3.enter_context(tc.tile_pool(name="lnsm", bufs=2))
                lnps = p3.enter_context(
                    tc.tile_pool(name="lnps", bufs=4, space="PSUM"))

                gamb = const.tile([128, H], F32, tag="gamb")
                betb = const.tile([128, H], F32, tag="betb")
                tmp1 = lnsm.tile([1, H], F32, tag="g1")
                nc.sync.dma_start(tmp1[:],
                                  gam_h.ap().rearrange("(a h) -> a h", a=1))
                nc.gpsimd.partition_broadcast(gamb[:], tmp1[:])
                tmp2 = lnsm.tile([1, H], F32, tag="g2")
                nc.sync.dma_start(tmp2[:],
                                  bet_h.ap().rearrange("(a h) -> a h", a=1))
                nc.gpsimd.partition_broadcast(betb[:], tmp2[:])
                epst = const.tile([128, 1], F32, tag="eps")
                nc.gpsimd.memset(epst[:], 1e-5)

                TG = 128 // B  # 32 steps per group
                for g in range(S // TG):
                    ldt = lnin.tile([128, TG, 4 * KH], BF16, tag="ldt")
                    nc.sync.dma_start(
                        ldt[:],
                        hraw_h[TG * g:TG * g + TG, :, :].rearrange(
                            "t p f -> p t f"))
                    # shuffle to ld[p, 128k + 32b + t] = ldt[p, t, 4k + b]
                    ld = lnin.tile([128, H], BF16, tag="ld")
                    nc.vector.tensor_copy(
                        ld[:].rearrange("p (k b t) -> p k b t", k=KH, b=B),
                        ldt[:].rearrange("p t (k b) -> p k b t", b=B))
                    row = lnt.tile([128, H], F32, tag="row")
                    for k in range(KH):
                        pl = lnps.tile([128, 128], BF16, tag="pl")
                        nc.tensor.transpose(
                            pl[:], ld[:, 128 * k:128 * k + 128], id_bf16[:])
                        nc.vector.tensor_copy(
                            row[:, 128 * k:128 * k + 128], pl[:])

                    # mean/var over H via bn_stats (two 512-wide subgroups)
                    stats = lnsm.tile([128, 2, 6], F32, tag="stats")
                    rowv = row[:].rearrange("p (a h) -> p a h", a=2)
                    for a in range(2):
                        nc.vector.bn_stats(stats[:, a, :], rowv[:, a, :])
                    mv = lnsm.tile([128, 2], F32, tag="mv")
                    nc.vector.bn_aggr(mv[:], stats[:])
                    rs = lnsm.tile([128, 1], F32, tag="rs")
                    nc.scalar.activation(rs[:], mv[:, 1:2], AF.Sqrt,
                                         bias=epst[:])
                    nc.vector.reciprocal(rs[:], rs[:])
                    # (x - mean) * rstd
                    nc.vector.tensor_scalar(
                        out=row[:], in0=row[:], scalar1=mv[:, 0:1],
                        scalar2=rs[:], op0=ALU.subtract, op1=ALU.mult)
                    nc.vector.tensor_tensor(row[:], row[:], gamb[:],
                                            op=ALU.mult)
                    nc.vector.tensor_tensor(row[:], row[:], betb[:],
                                            op=ALU.add)
                    for b in range(B):
                        nc.sync.dma_start(
                            out_h[b, TG * g:TG * g + TG, :],
                            row[TG * b:TG * b + TG, :])

    nc.compile()
    return nc


_NC_CACHE = {}


def _get_nc(S=S_FULL):
    if S not in _NC_CACHE:
        _NC_CACHE[S] = build_kernel(S)
    return _NC_CACHE[S]


def kernel(x, emb, W_ih, W_hh, b_ih, b_hh, gamma, beta):
    from concourse.bass_utils import run_bass_kernel_spmd

    x = np.asarray(x)
    S = x.shape[1]
    nc = _get_nc(S)

    common = {
        "emb": np.asarray(emb, dtype=np.float32),
        "W_ih": np.asarray(W_ih, dtype=np.float32),
        "W_hh": np.asarray(W_hh, dtype=np.float32),
        "b_ih": np.asarray(b_ih, dtype=np.float32),
        "b_hh": np.asarray(b_hh, dtype=np.float32),
        "gamma": np.asarray(gamma, dtype=np.float32),
        "beta": np.asarray(beta, dtype=np.float32),
    }
    in_maps = []
    for c in range(N_CORES):
        m = dict(common)
        m["x"] = np.asarray(x[B * c:B * (c + 1)], dtype=np.int32)
        in_maps.append(m)

    res = run_bass_kernel_spmd(nc, in_maps, core_ids=list(range(N_CORES)))
    outs = np.concatenate([r["out"] for r in res.results], axis=0)
    states = np.concatenate([r["state"] for r in res.results], axis=0)
    return outs, states


if __name__ == "__main__":
    nc = build_kernel()
    print("built + compiled ok")


# revision 20
# speedup vs baseline: 8.3430x; 8.3430x over previous
"""GRU encoder (embed -> GRU -> layernorm) Trainium2 Bass kernel.

Sharding: data-parallel over batch B=32 across 8 cores (4 rows each).
No collectives; each core runs an independent program on its batch slice.

Per-core pipeline:
  1. indirect-DMA gather of emb rows for the 2048 (b,s) tokens
  2. PE-transpose gathered rows + W_ih + W_hh into E/H-major bf16 layouts
  3. input GEMM xg = x_emb @ W_ih.T + biases  (bf16, gate-permuted cols r|n|z)
  4. 512 sequential GRU steps: hg = h @ W_hh.T via moving-W bf16 matmuls
     (lhsT = hT [128,4] chunks, rhs = W_hhT [128,512] chunks), gates on
     ACT/DVE, z/n transposed back to H-major via tiny PE transposes so the
     next step's lhsT needs no extra work
  5. final layernorm pass over the stored hidden states + output DMA
"""

import sys

sys.path.insert(0, "/opt/trn_rl_repo")

import numpy as np

import concourse.bass as bass
import concourse.tile as tile
from concourse import bacc, mybir
from concourse.masks import make_identity

F32 = mybir.dt.float32
BF16 = mybir.dt.bfloat16
I32 = mybir.dt.int32
AF = mybir.ActivationFunctionType
ALU = mybir.AluOpType

VOCAB, E, H = 32000, 512, 1024
B_FULL, S_FULL = 32, 512
N_CORES = 8
B = B_FULL // N_CORES  # 4 rows per core
G = 3 * H  # 3072 gate columns

# permuted gate-column layout: [r (0:1024) | n (1024:2048) | z (2048:3072)]
# (z last so the critical tail of each step is as short as possible)
R0, N0, Z0 = 0, H, 2 * H


def _perm_gchunk(c):
    """Map W row-chunk c (128 rows, original r,z,n order) to permuted col."""
    if c < 8:
        return 128 * c  # r
    if c < 16:
        return Z0 + 128 * (c - 8)  # z
    return N0 + 128 * (c - 16)  # n


# per 512-wide GEMM output chunk (original col order) -> permuted col base
_PERM_NCHUNK = [0, 512, Z0, Z0 + 512, N0, N0 + 512]


def build_kernel(S=S_FULL):
    nc = bacc.Bacc("TRN2", target_bir_lowering=False, debug=False,
                   enable_asserts=False)

    x_h = nc.dram_tensor("x", [B, S], I32, kind="ExternalInput")
    emb_h = nc.dram_tensor("emb", [VOCAB, E], F32, kind="ExternalInput")
    wih_h = nc.dram_tensor("W_ih", [G, E], F32, kind="ExternalInput")
    whh_h = nc.dram_tensor("W_hh", [G, H], F32, kind="ExternalInput")
    bih_h = nc.dram_tensor("b_ih", [G], F32, kind="ExternalInput")
    bhh_h = nc.dram_tensor("b_hh", [G], F32, kind="ExternalInput")
    gam_h = nc.dram_tensor("gamma", [H], F32, kind="ExternalInput")
    bet_h = nc.dram_tensor("beta", [H], F32, kind="ExternalInput")

    out_h = nc.dram_tensor("out", [B, S, H], F32, kind="ExternalOutput")
    st_h = nc.dram_tensor("state", [B, H], F32, kind="ExternalOutput")

    xg_h = nc.dram_tensor("xg_scratch", [B, S, G], BF16, kind="Internal")
    hraw_h = nc.dram_tensor("h_rawT", [S, 128, 4 * (H // 128)], BF16,
                            kind="Internal")

    NROWS = B * S          # (b, s) rows this core owns
    NRT = NROWS // 128     # 128-row tiles
    KE = E // 128          # 4  E-chunks
    KH = H // 128          # 8  H-chunks
    NG = G // 512          # 6  gate 512-chunks

    with tile.TileContext(nc) as tc:
        from contextlib import ExitStack

        with ExitStack() as ctx:
            const = ctx.enter_context(tc.tile_pool(name="const", bufs=1))

            # identities for PE transposes
            id_f32 = const.tile([128, 128], F32, tag="idf")
            make_identity(nc, id_f32[:])
            id_bf16 = const.tile([128, 128], BF16, tag="idb")
            nc.vector.tensor_copy(id_bf16[:], id_f32[:])

            # ones [1, B] bf16 for broadcasting b_hh_n into PSUM via matmul
            ones_b = const.tile([1, B], BF16, tag="ones")
            nc.gpsimd.memset(ones_b[:], 1.0)

            # biases: bias_perm[g'] = b_ih+b_hh (r,z) or b_ih (n), permuted
            braw = const.tile([1, 2 * G], F32, tag="braw")
            nc.sync.dma_start(braw[:, 0:G],
                              bih_h.ap().rearrange("(a g) -> a g", a=1))
            nc.sync.dma_start(braw[:, G:2 * G],
                              bhh_h.ap().rearrange("(a g) -> a g", a=1))
            bsum = const.tile([1, G], F32, tag="bsum")
            nc.vector.tensor_tensor(bsum[:], braw[:, 0:G], braw[:, G:2 * G],
                                    op=ALU.add)
            bperm = const.tile([1, G], F32, tag="bperm")
            nc.vector.tensor_copy(bperm[:, R0:R0 + H], bsum[:, 0:H])
            nc.vector.tensor_copy(bperm[:, Z0:Z0 + H], bsum[:, H:2 * H])
            nc.vector.tensor_copy(bperm[:, N0:N0 + H], braw[:, 2 * H:3 * H])
            # b_hh for the n gate (goes inside r*(...) so kept separate)
            bhhn = const.tile([1, H], BF16, tag="bhhn")
            nc.vector.tensor_copy(bhhn[:], braw[:, G + 2 * H:G + 3 * H])
            # broadcast bias across 128 partitions for the GEMM epilogue
            bias_bc = const.tile([128, G], F32, tag="biasbc")
            nc.gpsimd.partition_broadcast(bias_bc[:], bperm[:])

            # big persistent weight tile: W_hhT (H-major, permuted cols), bf16
            whhT = const.tile([128, KH * G], BF16, tag="whhT")

            # ---------------- phase 1: gather + transposes + GEMM ----------
            with ExitStack() as p1:
                gpool = p1.enter_context(tc.tile_pool(name="gath", bufs=3))
                tpsum = p1.enter_context(
                    tc.tile_pool(name="tpsum", bufs=4, space="PSUM"))
                big = p1.enter_context(tc.tile_pool(name="big", bufs=1))

                # token indices: idx[p, j] = x_flat[j*128 + p]
                idx = big.tile([128, NRT], I32, tag="idx")
                nc.gpsimd.dma_start(
                    idx[:],
                    x_h.ap().rearrange("b s -> (b s)").rearrange(
                        "(j p) -> p j", p=128))

                xembT = big.tile([128, KE * NROWS], BF16, tag="xembT")
                for j in range(NRT):
                    gt = gpool.tile([128, E], F32, tag="gt")
                    nc.gpsimd.indirect_dma_start(
                        out=gt[:],
                        out_offset=None,
                        in_=emb_h.ap(),
                        in_offset=bass.IndirectOffsetOnAxis(
                            ap=idx[:, j:j + 1], axis=0),
                    )
                    for e in range(KE):
                        pt = tpsum.tile([128, 128], F32, tag="pt")
                        nc.tensor.transpose(pt[:], gt[:, 128 * e:128 * e + 128],
                                            id_f32[:])
                        nc.vector.tensor_copy(
                            xembT[:, NROWS * e + 128 * j:
                                  NROWS * e + 128 * j + 128], pt[:])

                # W_ihT (E-major), bf16, original col order
                wihT = big.tile([128, KE * G], BF16, tag="wihT")
                for c in range(G // 128):
                    wt = gpool.tile([128, E], F32, tag="wt")
                    nc.sync.dma_start(wt[:], wih_h[128 * c:128 * c + 128, :])
                    for e in range(KE):
                        pt = tpsum.tile([128, 128], F32, tag="pt")
                        nc.tensor.transpose(pt[:], wt[:, 128 * e:128 * e + 128],
                                            id_f32[:])
                        nc.vector.tensor_copy(
                            wihT[:, G * e + 128 * c:G * e + 128 * c + 128],
                            pt[:])

                # GEMM: xg[rows, g] = x_emb @ W_ih.T  (+bias, permuted cols)
                gempsum = p1.enter_context(
                    tc.tile_pool(name="gemmps", bufs=4, space="PSUM"))
                xgout = p1.enter_context(tc.tile_pool(name="xgout", bufs=4))
                xg_rows = xg_h.ap().rearrange("b s g -> (b s) g")
                for rt in range(NRT):
                    for n in range(NG):
                        ps = gempsum.tile([128, 512], F32, tag="ps")
                        for e in range(KE):
                            nc.tensor.matmul(
                                ps[:],
                                lhsT=xembT[:, NROWS * e + 128 * rt:
                                           NROWS * e + 128 * rt + 128],
                                rhs=wihT[:, G * e + 512 * n:G * e + 512 * n + 512],
                                start=(e == 0), stop=(e == KE - 1))
                        pc = _PERM_NCHUNK[n]
                        xo = xgout.tile([128, 512], BF16, tag="xo")
                        nc.vector.tensor_tensor(
                            xo[:], ps[:], bias_bc[:, pc:pc + 512], op=ALU.add)
                        nc.sync.dma_start(
                            xg_rows[128 * rt:128 * rt + 128, pc:pc + 512],
                            xo[:])

                # W_hhT: H-major, permuted gate cols, bf16
                for c in range(G // 128):
                    wt2 = gpool.tile([128, H], F32, tag="wt2")
                    nc.sync.dma_start(wt2[:], whh_h[128 * c:128 * c + 128, :])
                    pcol = _perm_gchunk(c)
                    for k in range(KH):
                        pt = tpsum.tile([128, 128], F32, tag="pt")
                        nc.tensor.transpose(pt[:],
                                            wt2[:, 128 * k:128 * k + 128],
                                            id_f32[:])
                        nc.vector.tensor_copy(
                            whhT[:, G * k + pcol:G * k + pcol + 128], pt[:])

            # ---------------- phase 2: recurrence -------------------------
            with ExitStack() as p2:
                xgp = p2.enter_context(tc.tile_pool(name="xgp", bufs=4))
                prep = p2.enter_context(tc.tile_pool(name="prep", bufs=6))
                rznp = p2.enter_context(tc.tile_pool(name="rznp", bufs=2))
                htp = p2.enter_context(tc.tile_pool(name="htp", bufs=3))
                dep = p2.enter_context(tc.tile_pool(name="dep", bufs=3))
                psg = p2.enter_context(
                    tc.tile_pool(name="psg", bufs=6, space="PSUM"))
                pst = p2.enter_context(
                    tc.tile_pool(name="pst", bufs=2, space="PSUM"))

                hT = htp.tile([128, 4 * KH], BF16, tag="hT")
                nc.gpsimd.memset(hT[:], 0.0)

                for t in range(S):
                    xg = xgp.tile([B, G], BF16, tag="xg")
                    nc.sync.dma_start(
                        xg[:],
                        xg_h[:, t:t + 1, :].rearrange("b one g -> b (one g)"))

                    # hg psum chunks; col order r r n n z z (permuted layout)
                    ps = []
                    for n in range(NG):
                        p = psg.tile([B, 512], F32, tag="hgps")
                        ps.append(p)
                        first = True
                        if 512 * n >= N0 and 512 * n < Z0:
                            # n-gate chunk: seed with b_hh_n broadcast
                            off = 512 * n - N0
                            nc.tensor.matmul(
                                p[:], lhsT=ones_b[:],
                                rhs=bhhn[:, off:off + 512],
                                start=True, stop=False)
                            first = False
                        for k in range(KH):
                            nc.tensor.matmul(
                                p[:], lhsT=hT[:, 4 * k:4 * k + 4],
                                rhs=whhT[:, G * k + 512 * n:
                                         G * k + 512 * n + 512],
                                start=first and k == 0, stop=(k == KH - 1))

                    # r = sigmoid(hg_r + xg_r)   [B, 1024] bf16
                    r_sb = rznp.tile([B, H], BF16, tag="r")
                    for i in range(2):
                        pre = prep.tile([B, 512], F32, tag="pre")
                        nc.vector.tensor_tensor(
                            pre[:], ps[i][:], xg[:, R0 + 512 * i:R0 + 512 * i + 512],
                            op=ALU.add)
                        nc.scalar.activation(r_sb[:, 512 * i:512 * i + 512],
                                             pre[:], AF.Sigmoid)

                    # n = tanh(xg_n + r * (hg_n + b_hh_n))
                    n_sb = rznp.tile([B, H], BF16, tag="n")
                    for i in range(2):
                        t1 = prep.tile([B, 512], F32, tag="t1")
                        nc.vector.tensor_tensor(
                            t1[:], r_sb[:, 512 * i:512 * i + 512], ps[2 + i][:],
                            op=ALU.mult)
                        t2 = prep.tile([B, 512], F32, tag="t2")
                        nc.vector.tensor_tensor(
                            t2[:], t1[:], xg[:, N0 + 512 * i:N0 + 512 * i + 512],
                            op=ALU.add)
                        nc.scalar.activation(n_sb[:, 512 * i:512 * i + 512],
                                             t2[:], AF.Tanh)

                    # z = sigmoid(hg_z + xg_z)
                    z_sb = rznp.tile([B, H], BF16, tag="z")
                    for i in range(2):
                        pre = prep.tile([B, 512], F32, tag="prez")
                        nc.vector.tensor_tensor(
                            pre[:], ps[4 + i][:], xg[:, Z0 + 512 * i:Z0 + 512 * i + 512],
                            op=ALU.add)
                        nc.scalar.activation(z_sb[:, 512 * i:512 * i + 512],
                                             pre[:], AF.Sigmoid)

                    # transpose z, n back to H-major [128, 4*KH] (one shared
                    # psum bank: z in cols 0:32, n in cols 32:64)
                    psZN = pst.tile([128, 8 * KH], BF16, tag="psZN")
                    for c in range(KH):
                        nc.tensor.transpose(psZN[:, 4 * c:4 * c + 4],
                                            z_sb[:, 128 * c:128 * c + 128],
                                            id_bf16[0:B, 0:B])
                        nc.tensor.transpose(
                            psZN[:, 4 * KH + 4 * c:4 * KH + 4 * c + 4],
                            n_sb[:, 128 * c:128 * c + 128],
                            id_bf16[0:B, 0:B])
                    zT = dep.tile([128, 4 * KH], BF16, tag="zT")
                    nc.vector.tensor_copy(zT[:], psZN[:, 0:4 * KH])
                    nT = dep.tile([128, 4 * KH], BF16, tag="nT")
                    nc.scalar.copy(nT[:], psZN[:, 4 * KH:8 * KH])

                    # h' = n + z*(h - n)   (H-major)
                    d = dep.tile([128, 4 * KH], F32, tag="d")
                    nc.vector.tensor_tensor(d[:], hT[:], nT[:], op=ALU.subtract)
                    e_t = dep.tile([128, 4 * KH], F32, tag="e")
                    nc.vector.tensor_tensor(e_t[:], zT[:], d[:], op=ALU.mult)
                    hT = htp.tile([128, 4 * KH], BF16, tag="hT")
                    nc.vector.tensor_tensor(hT[:], nT[:], e_t[:], op=ALU.add)

                    nc.gpsimd.dma_start(
                        hraw_h[t:t + 1, :, :].rearrange("one p f -> p (one f)"),
                        hT[:])

                # final state: strided DRAM->DRAM cast from h_rawT[S-1]
                # st[b, 128k+p] = hraw[S-1, p, 4k+b]
                with nc.allow_non_contiguous_dma(reason="one-time 8KB state"):
                    for b in range(B):
                        nc.gpsimd.dma_start(
                            st_h[b:b + 1, :].rearrange(
                                "one (k p) -> (one p) k", p=128),
                            hraw_h[S - 1:S, :, :].rearrange(
                                "one p (k b) -> (one p) k b", b=B)[:, :, b])

            # ---------------- phase 3: layernorm + output ------------------
            with ExitStack() as p3:
                lnin = p3.enter_context(tc.tile_pool(name="lnin", bufs=2))
                lnt = p3.enter_context(tc.tile_pool(name="lnt", bufs=2))
                lnsm = p3.enter_context(tc.tile_pool(name="lnsm", bufs=2))
                lnps = p3.enter_context(
                    tc.tile_pool(name="lnps", bufs=4, space="PSUM"))

                gamb = const.tile([128, H], F32, tag="gamb")
                betb = const.tile([128, H], F32, tag="betb")
                tmp1 = lnsm.tile([1, H], F32, tag="g1")
                nc.sync.dma_start(tmp1[:],
                                  gam_h.ap().rearrange("(a h) -> a h", a=1))
                nc.gpsimd.partition_broadcast(gamb[:], tmp1[:])
                tmp2 = lnsm.tile([1, H], F32, tag="g2")
                nc.sync.dma_start(tmp2[:],
                                  bet_h.ap().rearrange("(a h) -> a h", a=1))
                nc.gpsimd.partition_broadcast(betb[:], tmp2[:])
                epst = const.tile([128, 1], F32, tag="eps")
                nc.gpsimd.memset(epst[:], 1e-5)

                TG = 128 // B  # 32 steps per group
                for g in range(S // TG):
                    ldt = lnin.tile([128, TG, 4 * KH], BF16, tag="ldt")
                    nc.sync.dma_start(
                        ldt[:],
                        hraw_h[TG * g:TG * g + TG, :, :].rearrange(
                            "t p f -> p t f"))
                    # shuffle to ld[p, 128k + 32b + t] = ldt[p, t, 4k + b]
                    ld = lnin.tile([128, H], BF16, tag="ld")
                    nc.vector.tensor_copy(
                        ld[:].rearrange("p (k b t) -> p k b t", k=KH, b=B),
                        ldt[:].rearrange("p t (k b) -> p k b t", b=B))
                    row = lnt.tile([128, H], F32, tag="row")
                    for k in range(KH):
                        pl = lnps.tile([128, 128], BF16, tag="pl")
                        nc.tensor.transpose(
                            pl[:], ld[:, 128 * k:128 * k + 128], id_bf16[:])
                        nc.vector.tensor_copy(
                            row[:, 128 * k:128 * k + 128], pl[:])

                    # mean/var over H via bn_stats (two 512-wide subgroups)
                    stats = lnsm.tile([128, 2, 6], F32, tag="stats")
                    rowv = row[:].rearrange("p (a h) -> p a h", a=2)
                    for a in range(2):
                        nc.vector.bn_stats(stats[:, a, :], rowv[:, a, :])
                    mv = lnsm.tile([128, 2], F32, tag="mv")
                    nc.vector.bn_aggr(mv[:], stats[:])
                    rs = lnsm.tile([128, 1], F32, tag="rs")
                    nc.scalar.activation(rs[:], mv[:, 1:2], AF.Sqrt,
                                         bias=epst[:])
                    nc.vector.reciprocal(rs[:], rs[:])
                    # (x - mean) * rstd
                    nc.vector.tensor_scalar(
                        out=row[:], in0=row[:], scalar1=mv[:, 0:1],
                        scalar2=rs[:], op0=ALU.subtract, op1=ALU.mult)
                    nc.vector.tensor_tensor(row[:], row[:], gamb[:],
                                            op=ALU.mult)
                    nc.vector.tensor_tensor(row[:], row[:], betb[:],
                                            op=ALU.add)
                    for b in range(B):
                        nc.sync.dma_start(
                            out_h[b, TG * g:TG * g + TG, :],
                            row[TG * b:TG * b + TG, :])

    nc.compile()
    return nc


_NC_CACHE = {}


def _get_nc(S=S_FULL):
    if S not in _NC_CACHE:
        _NC_CACHE[S] = build_kernel(S)
    return _NC_CACHE[S]


def kernel(x, emb, W_ih, W_hh, b_ih, b_hh, gamma, beta):
    from concourse.bass_utils import run_bass_kernel_spmd

    x = np.asarray(x)
    S = x.shape[1]
    nc = _get_nc(S)

    common = {
        "emb": np.asarray(emb, dtype=np.float32),
        "W_ih": np.asarray(W_ih, dtype=np.float32),
        "W_hh": np.asarray(W_hh, dtype=np.float32),
        "b_ih": np.asarray(b_ih, dtype=np.float32),
        "b_hh": np.asarray(b_hh, dtype=np.float32),
        "gamma": np.asarray(gamma, dtype=np.float32),
        "beta": np.asarray(beta, dtype=np.float32),
    }
    in_maps = []
    for c in range(N_CORES):
        m = dict(common)
        m["x"] = np.asarray(x[B * c:B * (c + 1)], dtype=np.int32)
        in_maps.append(m)

    res = run_bass_kernel_spmd(nc, in_maps, core_ids=list(range(N_CORES)))
    outs = np.concatenate([r["out"] for r in res.results], axis=0)
    states = np.concatenate([r["state"] for r in res.results], axis=0)
    return outs, states


if __name__ == "__main__":
    nc = build_kernel()
    print("built + compiled ok")
